# revision 1
# baseline (speedup 1.0000x reference)
"""Multi-head causal attention (B=2, T=2048, E=768, H=12, D=64) on 8 trn2 cores.

Sharding: core c handles batch b=c//4 and heads [3g, 3g+1, 3g+2] (g=c%4).
Each core computes its 3 heads' attention plus their partial contribution to the
final projection; the host sums the 4 partials per batch.

Per-core device program (all matmuls fp32r ~= tf32 precision):
  phase 1: qT/kT = (Wqk x^T + b)  four row-chunks [q0q1][k0k1][q2][k2]
           vT  = (Wv x^T + b)   [192, 2048] -> PE-transpose to v [s,d] tiles
                 with a ones column (for softmax denominators l = sum exp)
  phase 2: per head, per key block j: S^T = kT_j^T qT (K=64), +causal mask on
           diagonal block, P = exp(scale * S^T) (ACT, fused scale),
           [O^T; l] accumulated via matmul(lhsT=[v_j | 1], rhs=P).
           Normalize: recip = 1/l (DVE), partition-broadcast via K=1 matmul,
           elementwise multiply -> normalized O^T in SBUF.
  phase 3: out = sum_h O_h^T.T @ wf_h -> [2048, 768] partial, DMA out.

`repeat` unrolls the whole body N times in one NEFF; used by test.py to
measure per-body HW time as (t_N - t_1)/(N-1), cancelling dispatch overhead.
"""
import numpy as np

EMBED_DIM = 768
B = 2
T = 2048
N_CORES = 8
NT = T // 128           # 16 query/key tiles
SCALE = 1.0 / np.sqrt(64.0)
NEG = -1.0e9

_state = {}


def _build(repeat=1):
    import concourse.tile as tile
    from concourse import bacc, mybir
    from concourse.masks import make_identity

    F32 = mybir.dt.float32
    F32R = mybir.dt.float32r
    Exp = mybir.ActivationFunctionType.Exp
    ADD = mybir.AluOpType.add
    MULT = mybir.AluOpType.mult

    nc = bacc.Bacc("TRN2", target_bir_lowering=False, debug=False)

    BF16 = mybir.dt.bfloat16
    xT_d = nc.dram_tensor("xT", [EMBED_DIM, T], BF16, kind="ExternalInput").ap()
    # columns ordered [q0 q1 | k0 k1 | q2 | k2]
    wqk_d = nc.dram_tensor("wqk", [EMBED_DIM, 384], BF16, kind="ExternalInput").ap()
    wv_d = nc.dram_tensor("wv", [EMBED_DIM, 192], BF16, kind="ExternalInput").ap()
    bqk_d = nc.dram_tensor("bqk", [384, 1], F32, kind="ExternalInput").ap()
    bv_d = nc.dram_tensor("bv", [192, 1], F32, kind="ExternalInput").ap()
    wf_d = nc.dram_tensor("wf", [192, EMBED_DIM], F32R, kind="ExternalInput").ap()
    mask_d = nc.dram_tensor("mask", [128, 128], F32, kind="ExternalInput").ap()
    out_d = nc.dram_tensor("out_p", [T, EMBED_DIM], F32, kind="ExternalOutput").ap()

    bounds = [(0, 128), (128, 256), (256, 384)]

    with tile.TileContext(nc) as tc:
        with tc.tile_pool(name="const", bufs=1) as const, \
             tc.tile_pool(name="persist", bufs=1) as persist:
            # ---- constants ----
            BF16 = mybir.dt.bfloat16
            wqk_sb = const.tile([128, 6, 384], BF16)
            wv_sb = const.tile([128, 6, 192], BF16)
            nc.sync.dma_start(out=wqk_sb[:], in_=wqk_d.rearrange("(k p) c -> p k c", p=128))
            nc.scalar.dma_start(out=wv_sb[:], in_=wv_d.rearrange("(k p) c -> p k c", p=128))
            bqk_sb = [const.tile([128, 1], F32, name=f"bqk{m}", tag=f"bqk{m}")
                      for m in range(3)]
            for m, (r0, r1) in enumerate(bounds):
                nc.sync.dma_start(out=bqk_sb[m][:], in_=bqk_d[r0:r1, :])
            bv_sb = [const.tile([128, 1], F32, name="bv0", tag="bv0"),
                     const.tile([64, 1], F32, name="bv1", tag="bv1")]
            nc.sync.dma_start(out=bv_sb[0][:], in_=bv_d[0:128, :])
            nc.sync.dma_start(out=bv_sb[1][:], in_=bv_d[128:192, :])
            wf01_sb = const.tile([128, EMBED_DIM], F32R)
            wf2_sb = const.tile([64, EMBED_DIM], F32R)
            nc.scalar.dma_start(out=wf01_sb[:], in_=wf_d[0:128, :])
            nc.scalar.dma_start(out=wf2_sb[:], in_=wf_d[128:192, :])
            mask_sb = const.tile([128, 128], F32)
            nc.sync.dma_start(out=mask_sb[:], in_=mask_d[:])
            ident_f = const.tile([128, 128], F32)
            make_identity(nc, ident_f)
            ident_r = const.tile([128, 128], F32R)
            nc.vector.tensor_copy(out=ident_r[:], in_=ident_f[:])
            ones_f = const.tile([128, 64], F32)
            nc.vector.memset(ones_f[:], 1.0)
            ones_r = const.tile([128, 64], F32R)
            nc.vector.tensor_copy(out=ones_r[:], in_=ones_f[:])

            # ---- persistent activations ----
            qA = persist.tile([128, T], F32R)   # q0 @0:64, q1 @64:128
            kA = persist.tile([128, T], F32R)   # k0 @0:64, k1 @64:128
            qB = persist.tile([64, T], F32R)    # q2
            kB = persist.tile([64, T], F32R)    # k2
            v_all = persist.tile([128, NT, 3, 65], F32R)   # [v | 1] per head
            ot01 = persist.tile([128, T], F32R)  # normalized O^T heads 0 (@0) & 1 (@64)
            ot2 = persist.tile([64, T], F32R)    # head 2

            for rep in range(repeat):
                _emit_body(nc, tc, rep, locals())

    nc.compile()
    return nc


def _emit_body(nc, tc, rep, env):
    """Emit one full forward pass with interleaved emission so attention
    overlaps the tail of the projections.  (env var KPHASES limits emitted
    phases for benchmarking: "1", "12", or default "123")

    PSUM tags (8 banks): ps0 ps1 (projection groups + v-transposes, depth 2),
    st0 st1 st2 (S^T chunks), bc (1), otl (2 banks: [65, 1024] per (head,
    tq-half); also hosts phase-3 [128, 768] psum tiles)."""
    from concourse import mybir

    F32 = mybir.dt.float32
    F32R = mybir.dt.float32r
    Exp = mybir.ActivationFunctionType.Exp
    ADD = mybir.AluOpType.add
    MULT = mybir.AluOpType.mult

    xT_d, out_d = env["xT_d"], env["out_d"]
    wqk_sb, wv_sb = env["wqk_sb"], env["wv_sb"]
    bqk_sb, bv_sb = env["bqk_sb"], env["bv_sb"]
    wf01_sb, wf2_sb = env["wf01_sb"], env["wf2_sb"]
    ident_r, ones_f, ones_r = env["ident_r"], env["ones_f"], env["ones_r"]
    mask_sb = env["mask_sb"]
    qA, kA, qB, kB = env["qA"], env["kA"], env["qB"], env["kB"]
    v_all = env["v_all"]
    ot01, ot2 = env["ot01"], env["ot2"]
    dmae = [nc.sync, nc.scalar]

    with tc.tile_pool(name=f"sb{rep}", bufs=1) as sbp, \
         tc.tile_pool(name=f"ps{rep}", bufs=1, space="PSUM") as psp:
        # ---- input DMA: xT as 6x4 chunks spread over 3 DMA queues ----
        BF16 = mybir.dt.bfloat16
        xT_t = [[sbp.tile([128, 512], BF16, name=f"xT{rep}_{k}_{n}",
                          tag=f"xT{k}{n}") for n in range(4)] for k in range(6)]
        di = 0
        for n in range(4):
            for k in range(6):
                dmae[di % 2].dma_start(
                    out=xT_t[k][n][:],
                    in_=xT_d[128 * k:128 * (k + 1), 512 * n:512 * (n + 1)])
                di += 1

        gidx = [0]
        vT_sb = [sbp.tile([128, T], F32R, name=f"vT{rep}_0", tag="vT0"),
                 sbp.tile([64, T], F32R, name=f"vT{rep}_1", tag="vT1")]

        def qk_group(m, n):
            # m: 0=[q0q1]->qA, 1=[k0k1]->kA, 2=[q2|k2]->qB(0:64)+kB(64:128)
            c0, c1 = (0, 128) if m == 0 else (128, 256) if m == 1 else (256, 384)
            ps = psp.tile([128, 512], F32, name=f"pg{rep}_{gidx[0]}",
                          tag=f"ps{gidx[0] % 2}")
            gidx[0] += 1
            for k in range(6):
                nc.tensor.matmul(ps[:], lhsT=wqk_sb[:, k, c0:c1],
                                 rhs=xT_t[k][n][:], start=(k == 0), stop=(k == 5))
            nsl = slice(512 * n, 512 * (n + 1))
            if m < 2:
                dst = qA if m == 0 else kA
                nc.vector.tensor_scalar_add(out=dst[:, nsl], in0=ps[:],
                                            scalar1=bqk_sb[m][:])
            else:
                nc.vector.tensor_scalar_add(out=qB[:, nsl], in0=ps[0:64, :],
                                            scalar1=bqk_sb[2][0:64, :])
                nc.vector.tensor_scalar_add(out=kB[:, nsl], in0=ps[64:128, :],
                                            scalar1=bqk_sb[2][64:128, :])

        def v_group(m, n):
            pm = 128 if m == 0 else 64
            ps = psp.tile([128, 512], F32, name=f"pg{rep}_{gidx[0]}",
                          tag=f"ps{gidx[0] % 2}")
            gidx[0] += 1
            for k in range(6):
                nc.tensor.matmul(ps[:pm, :],
                                 lhsT=wv_sb[:, k, 128 * m:128 * m + pm],
                                 rhs=xT_t[k][n][:], start=(k == 0), stop=(k == 5))
            nc.vector.tensor_scalar_add(
                out=vT_sb[m][:pm, 512 * n:512 * (n + 1)],
                in0=ps[:pm, :], scalar1=bv_sb[m][:pm, :])

        def transposes(i):
            for h in range(3):
                m, off = divmod(64 * h, 128)
                tp = psp.tile([128, 64], F32R, name=f"tp{rep}_{i}{h}",
                              tag=f"ps{gidx[0] % 2}")
                gidx[0] += 1
                nc.tensor.transpose(
                    tp[:], vT_sb[m][off:off + 64, 128 * i:128 * (i + 1)],
                    ident_r[off:off + 64, off:off + 64])
                nc.vector.tensor_copy(out=v_all[:, i, h, 0:64], in_=tp[:])
            for h in range(3):
                nc.vector.tensor_copy(out=v_all[:, i, h, 64:65],
                                      in_=ones_f[:, 0:1])

        head_cfg = [(qA, kA, 0), (qA, kA, 64), (qB, kB, 0)]
        sidx = [0]
        recips = {}

        def attn_quarter(h, q):
            """Attention for head h restricted to tq in [512q, 512(q+1))."""
            qT, kT, o = head_cfg[h]
            if q == 0:
                recips[h] = sbp.tile([128, T], F32R, name=f"recip{rep}_{h}",
                                     tag=f"recip{h % 2}")
            recip = recips[h]
            otl = psp.tile([128, 512], F32, name=f"otl{rep}_{h}{q}",
                           tag=f"otl{(4 * h + q) % 2}")
            base = 512 * q
            jmax = 4 * q + 3
            items = []
            for j in range(jmax + 1):
                s0 = max(base, 128 * j)
                st = psp.tile([128, 512], F32, name=f"st{rep}_{h}{q}{j}",
                              tag=f"st{sidx[0] % 4}")
                pt = sbp.tile([128, 512], F32R, name=f"pt{rep}_{h}{q}{j}",
                              tag=f"pt{sidx[0] % 6}")
                sidx[0] += 1
                items.append((j, s0, base + 512 - s0, st, pt))
            for (j, s0, ln, st, pt) in items:
                nc.tensor.matmul(
                    st[:, 0:ln],
                    lhsT=kT[o:o + 64, 128 * j:128 * (j + 1)],
                    rhs=qT[o:o + 64, s0:s0 + ln],
                    start=True, stop=True)
                if s0 == 128 * j:
                    nc.vector.tensor_tensor(
                        out=st[:, 0:128], in0=st[:, 0:128],
                        in1=mask_sb[:], op=ADD)
            for (j, s0, ln, st, pt) in items:
                nc.scalar.activation(out=pt[:, 0:ln], in_=st[:, 0:ln],
                                     func=Exp, scale=float(SCALE))
            for (j, s0, ln, st, pt) in items:
                nc.tensor.matmul(
                    otl[0:65, s0 - base:512],
                    lhsT=v_all[:, j, h, :], rhs=pt[:, 0:ln],
                    start=(j == 0), stop=(j == jmax))
            # normalize: rows 0:64 of otl / row 64 (cross-base write for h1)
            ot_dst = ot2 if h == 2 else ot01
            ob_ = 64 if h == 1 else 0
            c0, c1 = base, base + 512
            with nc.allow_low_precision(reason="fp32r recip for bcast"):
                nc.vector.reciprocal(out=recip[64:65, c0:c1],
                                     in_=otl[64:65, :])
            bc = psp.tile([128, 512], F32, name=f"bc{rep}_{h}{q}", tag="ps0")
            nc.tensor.matmul(bc[0:64, :], lhsT=ones_r[64:65, :],
                             rhs=recip[64:65, c0:c1], start=True, stop=True)
            bcs = sbp.tile([128, 512], F32, name=f"bcs{rep}_{h}{q}",
                           tag=f"bcs{q % 2}")
            nc.vector.tensor_copy(out=bcs[0:64, :], in_=bc[0:64, :])
            nc.vector.tensor_tensor(
                out=ot_dst[ob_:ob_ + 64, c0:c1],
                in0=otl[0:64, :], in1=bcs[0:64, :], op=MULT)

        # ---- staged interleaved emission ----
        import os
        kphases = os.environ.get("KPHASES", "123")
        # A: minimal projections for heads 0/1 half 0
        for n in (0, 1):
            qk_group(0, n)
            qk_group(1, n)
            v_group(0, n)
            v_group(1, n)
        for i in range(0, 8):
            transposes(i)
        # B: head 0, first tq half (overlaps remaining projections)
        if "2" in kphases:
            attn_quarter(0, 0)
            attn_quarter(0, 1)
        # C: remaining projections
        for n in (2, 3):
            qk_group(0, n)
            qk_group(1, n)
        for n in range(4):
            qk_group(2, n)
        for n in (2, 3):
            v_group(0, n)
            v_group(1, n)
        for i in range(8, 16):
            transposes(i)
        # D-F: rest of attention
        if "2" in kphases:
            for (h, q) in [(0, 2), (0, 3), (1, 0), (1, 1), (1, 2), (1, 3),
                           (2, 0), (2, 1), (2, 2), (2, 3)]:
                attn_quarter(h, q)

        # ---------------- phase 3: output projection ----------------
        for i in range(NT if "3" in kphases else 1):
            fpa = psp.tile([128, 512], F32, name=f"fpa{rep}_{i}", tag=f"otl{i % 2}")
            fpb = psp.tile([128, 256], F32, name=f"fpb{rep}_{i}", tag=f"st{i % 4}")
            for (fp, n0, n1) in [(fpa, 0, 512), (fpb, 512, 768)]:
                nc.tensor.matmul(fp[:, 0:n1 - n0],
                                 lhsT=ot01[:, 128 * i:128 * (i + 1)],
                                 rhs=wf01_sb[:, n0:n1],
                                 start=True, stop=False)
                nc.tensor.matmul(fp[:, 0:n1 - n0],
                                 lhsT=ot2[:, 128 * i:128 * (i + 1)],
                                 rhs=wf2_sb[:, n0:n1],
                                 start=False, stop=True)
            ob = sbp.tile([128, EMBED_DIM], F32, name=f"ob{rep}_{i}",
                          tag=f"ob{i % 3}")
            nc.scalar.copy(out=ob[:, 0:512], in_=fpa[:, :])
            nc.vector.tensor_copy(out=ob[:, 512:768], in_=fpb[:, :])
            dmae[i % 2].dma_start(out=out_d[128 * i:128 * (i + 1), :], in_=ob[:])


def _prep_inputs(x, w_qkv, b_qkv, w_final):
    """Build the 8 per-core input maps from the full inputs."""
    x = np.asarray(x, dtype=np.float32)
    w_qkv = np.asarray(w_qkv, dtype=np.float32)
    b_qkv = np.asarray(b_qkv, dtype=np.float32)
    w_final = np.asarray(w_final, dtype=np.float32)
    E = EMBED_DIM

    mask = np.where(np.arange(128)[:, None] <= np.arange(128)[None, :], 0.0, NEG
                    ).astype(np.float32)
    in_maps = []
    for c in range(N_CORES):
        b = c // 4
        g = c % 4
        heads = [3 * g, 3 * g + 1, 3 * g + 2]
        hr = [np.arange(64 * h, 64 * h + 64) for h in heads]
        # [q0 q1 | k0 k1 | q2 | k2]
        rows_qk = np.concatenate([hr[0], hr[1], E + hr[0], E + hr[1], hr[2], E + hr[2]])
        rows_v = np.concatenate(hr) + 2 * E
        import ml_dtypes
        bf16 = ml_dtypes.bfloat16
        xT = np.ascontiguousarray(x[b].T).astype(bf16)          # [768, 2048]
        wqk = np.ascontiguousarray(w_qkv[rows_qk].T).astype(bf16)   # [768, 384]
        wv = np.ascontiguousarray(w_qkv[rows_v].T).astype(bf16)     # [768, 192]
        bqk = np.ascontiguousarray(b_qkv[rows_qk][:, None])
        bv = np.ascontiguousarray(b_qkv[rows_v][:, None])
        wf = np.ascontiguousarray(w_final[:, np.concatenate(hr)].T)  # [192, 768]
        in_maps.append({"xT": xT, "wqk": wqk, "wv": wv, "bqk": bqk, "bv": bv,
                        "wf": wf, "mask": mask})
    return in_maps


def kernel(x, w_qkv, b_qkv, w_final, _trace=False):
    from concourse.bass_utils import run_bass_kernel_spmd

    if "nc" not in _state:
        _state["nc"] = _build()
    nc = _state["nc"]

    in_maps = _prep_inputs(x, w_qkv, b_qkv, w_final)
    res = run_bass_kernel_spmd(nc, in_maps, list(range(N_CORES)), trace=_trace)
    _state["last_result"] = res

    out = np.empty((B, T, EMBED_DIM), dtype=np.float32)
    for b in range(B):
        acc = np.zeros((T, EMBED_DIM), dtype=np.float64)
        for g in range(4):
            acc += res.results[4 * b + g]["out_p"].astype(np.float64)
        out[b] = acc.astype(np.float32)
    return out



# revision 3
# speedup vs baseline: 1.5278x; 1.5278x over previous
"""Multi-head causal attention (B=2, T=2048, E=768, H=12, D=64) on 8 trn2 cores.

Sharding: core c handles batch b=c//4 and heads [3g, 3g+1, 3g+2] (g=c%4).
Each core computes its 3 heads' attention plus their partial contribution to
the final projection; the host sums the 4 partials per batch.

v2 redesign (vs v1):
- all matmuls bf16 (1 cyc/row at any moving size; no fp32r small-N penalty)
- v computed directly in [s, d] layout (no PE transposes); v bias folded into
  phase 3 via a constant ones row in ot2e and a host-precomputed bv@wf row
- causal mask added via PE matmul (ident^T @ mask) inside the S^T psum
  accumulation group instead of a DVE tensor_tensor
- exp instructions batched over pairs of key blocks (2-bank PSUM st tiles)
- DMA issue on sync + gpsimd queues (keeps Act SEQ free for exp)
- phase-3 ob moves split DVE (cols 0:512) / Act (cols 512:768)
- fine-grained emission interleave (attention vs projection/phase-3 filler)
  to keep the PE stream dense

Per-core program:
  phase 1: qT/kT = Wqk^T x^T + b, column groups [q0 q1][q2 k0][k1 k2];
           v[s,d] = x W_v^T per 128-query block (3 heads side by side)
  phase 2: per head h, key-block j: S^T_j = k_j^T q (K=64), +mask on diagonal
           blocks via matmul, P = exp(scale*S^T) (Act, pair-batched),
           [O^T; l] accumulated via matmul(lhsT=[v_j | 1], rhs=P).
           recip = 1/l (DVE), partition-broadcast via K=1 matmul, multiply.
  phase 3: out = [ot01; ot2e]^T @ [wf01; wf2e] -> [2048, 768] partial, DMA.

`repeat` unrolls the whole body N times in one NEFF; test.py measures
per-body HW time as the slope of wall time vs repeat count.
"""
import numpy as np

EMBED_DIM = 768
B = 2
T = 2048
N_CORES = 8
NT = T // 128           # 16 query/key tiles
SCALE = 1.0 / np.sqrt(64.0)
NEG = -1.0e9


_state = {}


def _build(repeat=1):
    import concourse.tile as tile
    from concourse import bacc, mybir
    from concourse.masks import make_identity

    F32 = mybir.dt.float32
    BF16 = mybir.dt.bfloat16

    nc = bacc.Bacc("TRN2", target_bir_lowering=False, debug=False)

    xT_d = nc.dram_tensor("xT", [EMBED_DIM, T], BF16, kind="ExternalInput").ap()
    # columns ordered [q0 q1 | k0 k1 | q2 k2]
    wqk_d = nc.dram_tensor("wqk", [EMBED_DIM, 384], BF16, kind="ExternalInput").ap()
    wv_d = nc.dram_tensor("wv", [EMBED_DIM, 192], BF16, kind="ExternalInput").ap()
    bqk_d = nc.dram_tensor("bqk", [384, 1], F32, kind="ExternalInput").ap()
    wf01_d = nc.dram_tensor("wf01", [128, EMBED_DIM], BF16, kind="ExternalInput").ap()
    wf2e_d = nc.dram_tensor("wf2e", [65, EMBED_DIM], BF16, kind="ExternalInput").ap()
    mask_d = nc.dram_tensor("mask", [128, 128], BF16, kind="ExternalInput").ap()
    out_d = nc.dram_tensor("out_p", [T, EMBED_DIM], BF16, kind="ExternalOutput").ap()

    with tile.TileContext(nc) as tc:
        with tc.tile_pool(name="const", bufs=1) as const, \
             tc.tile_pool(name="persist", bufs=1) as persist:
            # ---- constants ----
            wqk_sb = const.tile([128, 6, 384], BF16)
            wv_sb = const.tile([128, 6, 192], BF16)
            nc.sync.dma_start(out=wqk_sb[:], in_=wqk_d.rearrange("(k p) c -> p k c", p=128))
            nc.gpsimd.dma_start(out=wv_sb[:], in_=wv_d.rearrange("(k p) c -> p k c", p=128))
            bqk_sb = [const.tile([128, 1], F32, name=f"bqk{m}", tag=f"bqk{m}")
                      for m in range(3)]
            for m in range(3):
                nc.sync.dma_start(out=bqk_sb[m][:], in_=bqk_d[128 * m:128 * (m + 1), :])
            wf01_sb = const.tile([128, EMBED_DIM], BF16)
            wf2e_sb = const.tile([65, EMBED_DIM], BF16)
            nc.gpsimd.dma_start(out=wf01_sb[:], in_=wf01_d[:])
            nc.gpsimd.dma_start(out=wf2e_sb[:], in_=wf2e_d[:])
            mask_sb = const.tile([128, 128], BF16)
            nc.sync.dma_start(out=mask_sb[:], in_=mask_d[:])
            ident_f = const.tile([128, 128], F32)
            make_identity(nc, ident_f)
            ident_bf = const.tile([128, 128], BF16)
            nc.vector.tensor_copy(out=ident_bf[:], in_=ident_f[:])
            ones_bf = const.tile([65, 64], BF16)
            nc.vector.memset(ones_bf[:], 1.0)

            # ---- persistent activations ----
            qA = persist.tile([128, T], BF16)    # q0 @0:64, q1 @64:128
            kA = persist.tile([128, T], BF16)    # k0 @0:64, k1 @64:128
            qB = persist.tile([64, T], BF16)     # q2
            kB = persist.tile([64, T], BF16)     # k2
            v_all = persist.tile([128, NT, 3, 65], BF16)   # [v | 1] per head
            nc.vector.memset(v_all[:, :, :, 64:65], 1.0)
            ot01 = persist.tile([128, T], BF16)  # normalized O^T h0 (@0), h1 (@64)
            ot2e = persist.tile([65, T], BF16)   # h2 @0:64; row 64 = ones
            nc.vector.memset(ot2e[64:65, :], 1.0)

            with tc.tile_pool(name="sb", bufs=1) as sbp, \
                 tc.tile_pool(name="ps", bufs=1, space="PSUM") as psp:
                for rep in range(repeat):
                    _emit_body(nc, tc, rep, locals())

    nc.compile()
    return nc


def _emit_body(nc, tc, rep, env):
    """Emit one forward pass with fine-grained interleaving.

    PSUM tags (8 banks): ps0 ps1 (projection/v/bc groups), stp0 stp1
    (2-bank S^T pair tiles), otl0 otl1 (PV accumulators + phase-3)."""
    from concourse import mybir

    F32 = mybir.dt.float32
    BF16 = mybir.dt.bfloat16
    Exp = mybir.ActivationFunctionType.Exp
    MULT = mybir.AluOpType.mult

    xT_d, out_d = env["xT_d"], env["out_d"]
    wqk_sb, wv_sb = env["wqk_sb"], env["wv_sb"]
    bqk_sb = env["bqk_sb"]
    wf01_sb, wf2e_sb = env["wf01_sb"], env["wf2e_sb"]
    ident_bf, ones_bf, mask_sb = env["ident_bf"], env["ones_bf"], env["mask_sb"]
    qA, kA, qB, kB = env["qA"], env["kA"], env["qB"], env["kB"]
    v_all = env["v_all"]
    ot01, ot2e = env["ot01"], env["ot2e"]
    dmae = [nc.sync, nc.gpsimd]
    sbp, psp = env["sbp"], env["psp"]

    if True:
        # ---- input DMA: xT as 6x4 chunks spread over 2 DMA queues ----
        xT_t = [[sbp.tile([128, 512], BF16, name=f"xT{rep}_{k}_{n}",
                          tag=f"xT{k}{n}") for n in range(4)] for k in range(6)]
        di = 0
        for n in range(4):
            for k in range(6):
                dmae[di % 2].dma_start(
                    out=xT_t[k][n][:],
                    in_=xT_d[128 * k:128 * (k + 1), 512 * n:512 * (n + 1)])
                di += 1

        gidx = [0]

        def qk_group(m, n):
            ps = psp.tile([128, 512], F32, name=f"pg{rep}_{gidx[0]}",
                          tag=f"ps{gidx[0] % 2}")
            gidx[0] += 1
            for k in range(6):
                nc.tensor.matmul(ps[:], lhsT=wqk_sb[:, k, 128 * m:128 * (m + 1)],
                                 rhs=xT_t[k][n][:], start=(k == 0), stop=(k == 5))
            nsl = slice(512 * n, 512 * (n + 1))
            if m < 2:
                dst = qA if m == 0 else kA
                nc.vector.tensor_scalar_add(out=dst[:, nsl], in0=ps[:],
                                            scalar1=bqk_sb[m][:])
            else:
                nc.vector.tensor_scalar_add(out=qB[:, nsl], in0=ps[0:64, :],
                                            scalar1=bqk_sb[2][0:64, :])
                nc.vector.tensor_scalar_add(out=kB[:, nsl], in0=ps[64:128, :],
                                            scalar1=bqk_sb[2][64:128, :])

        def v_block(i):
            # v[s, d] for s-block i, 3 heads side by side: [128, 192]
            n, off = divmod(128 * i, 512)
            ps = psp.tile([128, 512], F32, name=f"vp{rep}_{i}",
                          tag=f"ps{gidx[0] % 2}")
            gidx[0] += 1
            for k in range(6):
                nc.tensor.matmul(ps[:, 0:192], lhsT=xT_t[k][n][:, off:off + 128],
                                 rhs=wv_sb[:, k, :], start=(k == 0), stop=(k == 5))
            nc.vector.tensor_copy(
                out=v_all[:, i, :, 0:64],
                in_=ps[:, 0:192].rearrange("p (h d) -> p h d", h=3))

        # h -> (qT tile, q part offset, kT tile, k part offset)
        head_cfg = [(qA, 0, kA, 0), (qA, 64, kA, 64), (qB, 0, kB, 0)]
        sidx = [0]
        otli = [0]
        attn_st = {}

        def attn_S(h, q, inject=None):
            """S^T matmuls + diag mask + pair-batched exp for quarter q.
            inject: {pair_index: item} emitted after that pair, to absorb the
            exp pipeline phase lag without displacing the S stream."""
            qT, oq, kT, ok = head_cfg[h]
            base = 512 * q
            pairs = []
            for p in range(2 * q + 2):
                if inject and p in inject:
                    inject.pop(p)()
                st2 = psp.tile([128, 2, 512], F32, name=f"st{rep}_{h}{q}{p}",
                               tag=f"stp{sidx[0] % 2}")
                pt2 = sbp.tile([128, 2, 512], BF16, name=f"pt{rep}_{h}{q}{p}",
                               tag=f"pt{sidx[0] % 6}")
                sidx[0] += 1
                lns = []
                for jj in range(2):
                    j = 2 * p + jj
                    s0 = max(base, 128 * j)
                    ln = base + 512 - s0
                    lns.append(ln)
                    diag = 128 * j >= base
                    nc.tensor.matmul(
                        st2[:, jj, 0:ln],
                        lhsT=kT[ok:ok + 64, 128 * j:128 * (j + 1)],
                        rhs=qT[oq:oq + 64, s0:s0 + ln],
                        start=True, stop=not diag)
                    if diag:
                        nc.tensor.matmul(
                            st2[:, jj, 0:128], lhsT=ident_bf[:], rhs=mask_sb[:],
                            start=False, stop=True)
                mx = max(lns)
                nc.scalar.activation(out=pt2[:, :, 0:mx], in_=st2[:, :, 0:mx],
                                     func=Exp, scale=float(SCALE))
                pairs.append((st2, pt2, lns))
            attn_st[(h, q)] = pairs

        def attn_PV(h, q):
            base = 512 * q
            pairs = attn_st.pop((h, q))
            otl = psp.tile([128, 512], F32, name=f"otl{rep}_{h}{q}",
                           tag=f"otl{otli[0] % 2}")
            otli[0] += 1
            jmax = 4 * q + 3
            for j in range(jmax + 1):
                s0 = max(base, 128 * j)
                ln = base + 512 - s0
                pt2 = pairs[j // 2][1]
                nc.tensor.matmul(
                    otl[0:65, s0 - base:512],
                    lhsT=v_all[:, j, h, :], rhs=pt2[:, j % 2, 0:ln],
                    start=(j == 0), stop=(j == jmax))
            return otl

        def attn_norm(h, q, otl):
            base = 512 * q
            recip = sbp.tile([65, 512], F32, name=f"rc{rep}_{h}{q}",
                             tag=f"rc{(3 * q + h) % 2}")
            nc.vector.reciprocal(out=recip[0:1, :], in_=otl[64:65, :])
            bcs = sbp.tile([64, 512], F32, name=f"bcs{rep}_{h}{q}",
                           tag=f"bcs{(3 * q + h) % 2}")
            nc.gpsimd.partition_broadcast(bcs[:, :], recip[0:1, :])
            dst, od = (ot2e, 0) if h == 2 else (ot01, 64 * h)
            nc.vector.tensor_tensor(
                out=dst[od:od + 64, base:base + 512],
                in0=otl[0:64, :], in1=bcs[:], op=MULT)

        p3_ob = {}

        def phase3a(i):
            fpa = psp.tile([128, 512], F32, name=f"fpa{rep}_{i}", tag=f"otl{i % 2}")
            ti = slice(128 * i, 128 * (i + 1))
            nc.tensor.matmul(fpa[:], lhsT=ot01[:, ti],
                             rhs=wf01_sb[:, 0:512], start=True, stop=False)
            nc.tensor.matmul(fpa[:], lhsT=ot2e[:, ti],
                             rhs=wf2e_sb[:, 0:512], start=False, stop=True)
            ob = sbp.tile([128, EMBED_DIM], BF16, name=f"ob{rep}_{i}",
                          tag=f"ob{i % 6}")
            p3_ob[i] = ob
            nc.vector.tensor_copy(out=ob[:, 0:512], in_=fpa[:, :])

        def phase3b(i):
            fpb = psp.tile([128, 512], F32, name=f"fpb{rep}_{i}", tag=f"otl{i % 2}")
            ti = slice(128 * i, 128 * (i + 1))
            nc.tensor.matmul(fpb[:, 0:256], lhsT=ot01[:, ti],
                             rhs=wf01_sb[:, 512:768], start=True, stop=False)
            nc.tensor.matmul(fpb[:, 0:256], lhsT=ot2e[:, ti],
                             rhs=wf2e_sb[:, 512:768], start=False, stop=True)
            ob = p3_ob.pop(i)
            nc.vector.tensor_copy(out=ob[:, 512:768], in_=fpb[:, 0:256])
            dmae[i % 2].dma_start(out=out_d[ti, :], in_=ob[:])

        # ---- staged emission ----
        # Safety rule: attention quarters are emitted as contiguous blocks
        # (their st/pt/otl tag rotations + cross-engine deps form cycles if
        # other otl-tag users are woven in).  Projections and phase-3 use
        # disjoint tag sets, so they can interleave with each other freely.
        def attn_quarter(q, prelude=(), inject=None):
            """Attention for quarter q, inter-head pipelined.  `prelude` items
            (phase-3 leftovers) are emitted after the first S batch so their
            psum-tag waits hide behind this quarter's exp latency."""
            otl = {}
            attn_S(0, q, inject)
            for it in prelude:
                it()
            attn_S(1, q)
            otl[0] = attn_PV(0, q)
            attn_S(2, q)
            otl[1] = attn_PV(1, q)
            attn_norm(0, q, otl.pop(0))
            otl[2] = attn_PV(2, q)
            attn_norm(1, q, otl.pop(1))
            attn_norm(2, q, otl.pop(2))

        def proj_items(n):
            items = [lambda m=m: qk_group(m, n) for m in range(3)]
            items += [lambda i=i: v_block(i) for i in range(4 * n, 4 * n + 4)]
            return items

        def phase3_items(i0, i1, defer_tail=False):
            """a/b sub-items pipelined: a(i), a(i+1), b(i), a(i+2), b(i+1)...
            With defer_tail, the trailing b-items are split off for the
            caller to emit later."""
            out = []
            pend = []
            for i in range(i0, i1):
                out.append(lambda i=i: phase3a(i))
                pend.append(lambda i=i: phase3b(i))
                if len(pend) > 1:
                    out.append(pend.pop(0))
            if defer_tail:
                return out, pend
            out.extend(pend)
            return out

        def interleave(a_items, b_items):
            out = []
            na, nb = len(a_items), len(b_items)
            bi = 0
            for ai, item in enumerate(a_items):
                out.append(item)
                want = int(round((ai + 1) * nb / na))
                while bi < want:
                    out.append(b_items[bi])
                    bi += 1
            out.extend(b_items[bi:])
            return out

        for it in proj_items(0) + proj_items(1):
            it()
        attn_quarter(0)
        p2 = proj_items(2)
        vb8 = p2.pop(3)
        for it in p2:
            it()
        attn_quarter(1, inject={2: vb8})
        p3i = proj_items(3)
        vb12 = p3i.pop(3)
        for it in interleave(p3i, phase3_items(0, 4)):
            it()
        attn_quarter(2, inject={2: vb12})
        items, tail = phase3_items(4, 8, defer_tail=True)
        for it in items:
            it()
        attn_quarter(3, prelude=tail)
        for it in phase3_items(8, 16):
            it()


# revision 4
# speedup vs baseline: 2.1654x; 1.4173x over previous
"""Multi-head causal attention (B=2, T=2048, E=768, H=12, D=64) on 8 trn2 cores.

Sharding: core c handles batch b=c//4 and heads [3g, 3g+1, 3g+2] (g=c%4).
Each core computes its 3 heads' attention plus their partial contribution to
the final projection; the host sums the 4 partials per batch.

v2 redesign (vs v1):
- all matmuls bf16 (1 cyc/row at any moving size; no fp32r small-N penalty)
- v computed directly in [s, d] layout (no PE transposes); v bias folded into
  phase 3 via a constant ones row in ot2e and a host-precomputed bv@wf row
- causal mask added via PE matmul (ident^T @ mask) inside the S^T psum
  accumulation group instead of a DVE tensor_tensor
- exp instructions batched over pairs of key blocks (2-bank PSUM st tiles)
- DMA issue on sync + gpsimd queues (keeps Act SEQ free for exp)
- phase-3 ob moves split DVE (cols 0:512) / Act (cols 512:768)
- fine-grained emission interleave (attention vs projection/phase-3 filler)
  to keep the PE stream dense

Per-core program:
  phase 1: qT/kT = Wqk^T x^T + b, column groups [q0 q1][q2 k0][k1 k2];
           v[s,d] = x W_v^T per 128-query block (3 heads side by side)
  phase 2: per head h, key-block j: S^T_j = k_j^T q (K=64), +mask on diagonal
           blocks via matmul, P = exp(scale*S^T) (Act, pair-batched),
           [O^T; l] accumulated via matmul(lhsT=[v_j | 1], rhs=P).
           recip = 1/l (DVE), partition-broadcast via K=1 matmul, multiply.
  phase 3: out = [ot01; ot2e]^T @ [wf01; wf2e] -> [2048, 768] partial, DMA.

`repeat` unrolls the whole body N times in one NEFF; test.py measures
per-body HW time as the slope of wall time vs repeat count.
"""
import numpy as np

EMBED_DIM = 768
B = 2
T = 2048
N_CORES = 8
NT = T // 128           # 16 query/key tiles
SCALE = 1.0 / np.sqrt(64.0)
NEG = -1.0e9


_state = {}


def _build(repeat=1):
    import concourse.tile as tile
    from concourse import bacc, mybir
    from concourse.masks import make_identity

    F32 = mybir.dt.float32
    BF16 = mybir.dt.bfloat16

    nc = bacc.Bacc("TRN2", target_bir_lowering=False, debug=False)

    xT_d = nc.dram_tensor("xT", [EMBED_DIM, T], BF16, kind="ExternalInput").ap()
    # columns ordered [q0 q1 | k0 k1 | q2 k2]
    wqk_d = nc.dram_tensor("wqk", [EMBED_DIM, 384], BF16, kind="ExternalInput").ap()
    wv_d = nc.dram_tensor("wv", [EMBED_DIM, 192], BF16, kind="ExternalInput").ap()
    bqk_d = nc.dram_tensor("bqk", [384, 1], F32, kind="ExternalInput").ap()
    wf01_d = nc.dram_tensor("wf01", [128, EMBED_DIM], BF16, kind="ExternalInput").ap()
    wf2e_d = nc.dram_tensor("wf2e", [65, EMBED_DIM], BF16, kind="ExternalInput").ap()
    mask_d = nc.dram_tensor("mask", [128, 128], BF16, kind="ExternalInput").ap()
    out_d = nc.dram_tensor("out_p", [T, EMBED_DIM], BF16, kind="ExternalOutput").ap()

    with tile.TileContext(nc) as tc:
        with tc.tile_pool(name="const", bufs=1) as const, \
             tc.tile_pool(name="persist", bufs=1) as persist:
            # ---- constants ----
            wqk_sb = const.tile([128, 6, 384], BF16)
            wv_sb = const.tile([128, 6, 192], BF16)
            nc.sync.dma_start(out=wqk_sb[:], in_=wqk_d.rearrange("(k p) c -> p k c", p=128))
            nc.gpsimd.dma_start(out=wv_sb[:], in_=wv_d.rearrange("(k p) c -> p k c", p=128))
            bqk_sb = [const.tile([128, 1], F32, name=f"bqk{m}", tag=f"bqk{m}")
                      for m in range(3)]
            for m in range(3):
                nc.sync.dma_start(out=bqk_sb[m][:], in_=bqk_d[128 * m:128 * (m + 1), :])
            wf01_sb = const.tile([128, EMBED_DIM], BF16)
            wf2e_sb = const.tile([65, EMBED_DIM], BF16)
            nc.gpsimd.dma_start(out=wf01_sb[:], in_=wf01_d[:])
            nc.gpsimd.dma_start(out=wf2e_sb[:], in_=wf2e_d[:])
            mask_sb = const.tile([128, 128], BF16)
            nc.sync.dma_start(out=mask_sb[:], in_=mask_d[:])
            ident_f = const.tile([128, 128], F32)
            make_identity(nc, ident_f)
            ident_bf = const.tile([128, 128], BF16)
            nc.vector.tensor_copy(out=ident_bf[:], in_=ident_f[:])
            ones_bf = const.tile([65, 64], BF16)
            nc.vector.memset(ones_bf[:], 1.0)

            # ---- persistent activations ----
            qA = persist.tile([128, T], BF16)    # q0 @0:64, q1 @64:128
            kA = persist.tile([128, T], BF16)    # k0 @0:64, k1 @64:128
            qB = persist.tile([64, T], BF16)     # q2
            kB = persist.tile([64, T], BF16)     # k2
            v_all = persist.tile([128, NT, 3, 65], BF16)   # [v | 1] per head
            nc.vector.memset(v_all[:, :, :, 64:65], 1.0)
            ot01 = persist.tile([128, T], BF16)  # normalized O^T h0 (@0), h1 (@64)
            ot2e = persist.tile([65, T], BF16)   # h2 @0:64; row 64 = ones
            nc.vector.memset(ot2e[64:65, :], 1.0)

            with tc.tile_pool(name="sb", bufs=1) as sbp, \
                 tc.tile_pool(name="ps", bufs=1, space="PSUM") as psp:
                for rep in range(repeat):
                    _emit_body(nc, tc, rep, locals())

    nc.compile()
    return nc


def _emit_body(nc, tc, rep, env):
    """Emit one forward pass with fine-grained interleaving.

    PSUM tags (8 banks): ps0 ps1 (projection/v/bc groups), stp0 stp1
    (2-bank S^T pair tiles), otl0 otl1 (PV accumulators + phase-3)."""
    from concourse import mybir

    F32 = mybir.dt.float32
    BF16 = mybir.dt.bfloat16
    Exp = mybir.ActivationFunctionType.Exp
    MULT = mybir.AluOpType.mult

    xT_d, out_d = env["xT_d"], env["out_d"]
    wqk_sb, wv_sb = env["wqk_sb"], env["wv_sb"]
    bqk_sb = env["bqk_sb"]
    wf01_sb, wf2e_sb = env["wf01_sb"], env["wf2e_sb"]
    ident_bf, ones_bf, mask_sb = env["ident_bf"], env["ones_bf"], env["mask_sb"]
    qA, kA, qB, kB = env["qA"], env["kA"], env["qB"], env["kB"]
    v_all = env["v_all"]
    ot01, ot2e = env["ot01"], env["ot2e"]
    dmae = [nc.sync, nc.gpsimd]
    sbp, psp = env["sbp"], env["psp"]

    if True:
        # ---- input DMA: xT as 6x4 chunks spread over 2 DMA queues ----
        xT_t = [[sbp.tile([128, 512], BF16, name=f"xT{rep}_{k}_{n}",
                          tag=f"xT{k}{n}") for n in range(4)] for k in range(6)]
        di = 0
        for n in range(4):
            for k in range(6):
                dmae[di % 2].dma_start(
                    out=xT_t[k][n][:],
                    in_=xT_d[128 * k:128 * (k + 1), 512 * n:512 * (n + 1)])
                di += 1

        gidx = [0]

        def qk_group(m, n):
            ps = psp.tile([128, 512], F32, name=f"pg{rep}_{gidx[0]}",
                          tag=f"ps{gidx[0] % 2}")
            gidx[0] += 1
            for k in range(6):
                nc.tensor.matmul(ps[:], lhsT=wqk_sb[:, k, 128 * m:128 * (m + 1)],
                                 rhs=xT_t[k][n][:], start=(k == 0), stop=(k == 5))
            nsl = slice(512 * n, 512 * (n + 1))
            if m < 2:
                dst = qA if m == 0 else kA
                nc.vector.tensor_scalar_add(out=dst[:, nsl], in0=ps[:],
                                            scalar1=bqk_sb[m][:])
            else:
                nc.vector.tensor_scalar_add(out=qB[:, nsl], in0=ps[0:64, :],
                                            scalar1=bqk_sb[2][0:64, :])
                nc.vector.tensor_scalar_add(out=kB[:, nsl], in0=ps[64:128, :],
                                            scalar1=bqk_sb[2][64:128, :])

        def v_block(i):
            # v[s, d] for s-block i, 3 heads side by side: [128, 192]
            n, off = divmod(128 * i, 512)
            ps = psp.tile([128, 512], F32, name=f"vp{rep}_{i}",
                          tag=f"ps{gidx[0] % 2}")
            gidx[0] += 1
            for k in range(6):
                nc.tensor.matmul(ps[:, 0:192], lhsT=xT_t[k][n][:, off:off + 128],
                                 rhs=wv_sb[:, k, :], start=(k == 0), stop=(k == 5))
            nc.vector.tensor_copy(
                out=v_all[:, i, :, 0:64],
                in_=ps[:, 0:192].rearrange("p (h d) -> p h d", h=3))

        # h -> (qT tile, q part offset, kT tile, k part offset)
        head_cfg = [(qA, 0, kA, 0), (qA, 64, kA, 64), (qB, 0, kB, 0)]
        sidx = [0]
        otli = [0]
        attn_st = {}

        def attn_S(h, q, inject=None):
            """S^T matmuls + diag mask + pair-batched exp for quarter q.
            inject: {pair_index: item} emitted after that pair, to absorb the
            exp pipeline phase lag without displacing the S stream."""
            qT, oq, kT, ok = head_cfg[h]
            base = 512 * q
            pairs = []
            for p in range(2 * q + 2):
                if inject and p in inject:
                    inject.pop(p)()
                st2 = psp.tile([128, 2, 512], F32, name=f"st{rep}_{h}{q}{p}",
                               tag=f"stp{sidx[0] % 2}")
                pt2 = sbp.tile([128, 2, 512], BF16, name=f"pt{rep}_{h}{q}{p}",
                               tag=f"pt{sidx[0] % 6}")
                sidx[0] += 1
                lns = []
                diags = []
                for jj in range(2):
                    j = 2 * p + jj
                    s0 = max(base, 128 * j)
                    ln = base + 512 - s0
                    lns.append(ln)
                    if 128 * j >= base:
                        diags.append(jj)
                    nc.tensor.matmul(
                        st2[:, jj, 0:ln],
                        lhsT=kT[ok:ok + 64, 128 * j:128 * (j + 1)],
                        rhs=qT[oq:oq + 64, s0:s0 + ln],
                        start=True, stop=True)
                mx = max(lns)
                nc.scalar.activation(out=pt2[:, :, 0:mx], in_=st2[:, :, 0:mx],
                                     func=Exp, scale=float(SCALE))
                for jj in diags:
                    nc.vector.tensor_tensor(
                        out=pt2[:, jj, 0:128], in0=pt2[:, jj, 0:128],
                        in1=mask_sb[:], op=MULT)
                pairs.append((st2, pt2, lns))
            attn_st[(h, q)] = pairs

        def attn_PV(h, q):
            base = 512 * q
            pairs = attn_st.pop((h, q))
            otl = psp.tile([128, 512], F32, name=f"otl{rep}_{h}{q}",
                           tag=f"otl{otli[0] % 2}")
            otli[0] += 1
            jmax = 4 * q + 3
            for j in range(jmax + 1):
                s0 = max(base, 128 * j)
                ln = base + 512 - s0
                pt2 = pairs[j // 2][1]
                nc.tensor.matmul(
                    otl[0:65, s0 - base:512],
                    lhsT=v_all[:, j, h, :], rhs=pt2[:, j % 2, 0:ln],
                    start=(j == 0), stop=(j == jmax))
            return otl

        def attn_norm(h, q, otl):
            base = 512 * q
            recip = sbp.tile([65, 512], F32, name=f"rc{rep}_{h}{q}",
                             tag=f"rc{(3 * q + h) % 2}")
            nc.vector.reciprocal(out=recip[0:1, :], in_=otl[64:65, :])
            bcs = sbp.tile([64, 512], F32, name=f"bcs{rep}_{h}{q}",
                           tag=f"bcs{(3 * q + h) % 2}")
            nc.gpsimd.partition_broadcast(bcs[:, :], recip[0:1, :])
            dst, od = (ot2e, 0) if h == 2 else (ot01, 64 * h)
            nc.vector.tensor_tensor(
                out=dst[od:od + 64, base:base + 512],
                in0=otl[0:64, :], in1=bcs[:], op=MULT)

        p3_ob = {}

        def phase3a(i):
            fpa = psp.tile([128, 512], F32, name=f"fpa{rep}_{i}", tag=f"otl{i % 2}")
            ti = slice(128 * i, 128 * (i + 1))
            nc.tensor.matmul(fpa[:], lhsT=ot01[:, ti],
                             rhs=wf01_sb[:, 0:512], start=True, stop=False)
            nc.tensor.matmul(fpa[:], lhsT=ot2e[:, ti],
                             rhs=wf2e_sb[:, 0:512], start=False, stop=True)
            ob = sbp.tile([128, EMBED_DIM], BF16, name=f"ob{rep}_{i}",
                          tag=f"ob{i % 6}")
            p3_ob[i] = ob
            nc.vector.tensor_copy(out=ob[:, 0:512], in_=fpa[:, :])

        def phase3b(i):
            fpb = psp.tile([128, 512], F32, name=f"fpb{rep}_{i}", tag=f"otl{i % 2}")
            ti = slice(128 * i, 128 * (i + 1))
            nc.tensor.matmul(fpb[:, 0:256], lhsT=ot01[:, ti],
                             rhs=wf01_sb[:, 512:768], start=True, stop=False)
            nc.tensor.matmul(fpb[:, 0:256], lhsT=ot2e[:, ti],
                             rhs=wf2e_sb[:, 512:768], start=False, stop=True)
            ob = p3_ob.pop(i)
            nc.vector.tensor_copy(out=ob[:, 512:768], in_=fpb[:, 0:256])
            dmae[i % 2].dma_start(out=out_d[ti, :], in_=ob[:])

        # ---- staged emission ----
        # Safety rule: attention quarters are emitted as contiguous blocks
        # (their st/pt/otl tag rotations + cross-engine deps form cycles if
        # other otl-tag users are woven in).  Projections and phase-3 use
        # disjoint tag sets, so they can interleave with each other freely.
        def attn_quarter(q, prelude=(), inject=None):
            """Attention for quarter q, inter-head pipelined.  `prelude` items
            (phase-3 leftovers) are emitted after the first S batch so their
            psum-tag waits hide behind this quarter's exp latency."""
            otl = {}
            attn_S(0, q, inject)
            for it in prelude:
                it()
            attn_S(1, q)
            otl[0] = attn_PV(0, q)
            attn_S(2, q)
            otl[1] = attn_PV(1, q)
            attn_norm(0, q, otl.pop(0))
            otl[2] = attn_PV(2, q)
            attn_norm(1, q, otl.pop(1))
            attn_norm(2, q, otl.pop(2))

        def proj_items(n):
            items = [lambda m=m: qk_group(m, n) for m in range(3)]
            items += [lambda i=i: v_block(i) for i in range(4 * n, 4 * n + 4)]
            return items

        def phase3_items(i0, i1, defer_tail=False):
            """a/b sub-items pipelined: a(i), a(i+1), b(i), a(i+2), b(i+1)...
            With defer_tail, the trailing b-items are split off for the
            caller to emit later."""
            out = []
            pend = []
            for i in range(i0, i1):
                out.append(lambda i=i: phase3a(i))
                pend.append(lambda i=i: phase3b(i))
                if len(pend) > 1:
                    out.append(pend.pop(0))
            if defer_tail:
                return out, pend
            out.extend(pend)
            return out

        def interleave(a_items, b_items):
            out = []
            na, nb = len(a_items), len(b_items)
            bi = 0
            for ai, item in enumerate(a_items):
                out.append(item)
                want = int(round((ai + 1) * nb / na))
                while bi < want:
                    out.append(b_items[bi])
                    bi += 1
            out.extend(b_items[bi:])
            return out

        for it in proj_items(0) + proj_items(1):
            it()
        attn_quarter(0)
        p2 = proj_items(2)
        vb8 = p2.pop(3)
        for it in p2:
            it()
        attn_quarter(1, inject={2: vb8})
        p3i = proj_items(3)
        vb12 = p3i.pop(3)
        for it in interleave(p3i, phase3_items(0, 4)):
            it()
        attn_quarter(2, inject={2: vb12})
        items, tail = phase3_items(4, 8, defer_tail=True)
        for it in items:
            it()
        attn_quarter(3, prelude=tail)
        for it in phase3_items(8, 16):
            it()


# revision 5
# speedup vs baseline: 2.2045x; 1.0181x over previous
"""Multi-head causal attention (B=2, T=2048, E=768, H=12, D=64) on 8 trn2 cores.

Sharding: core c handles batch b=c//4 and heads [3g, 3g+1, 3g+2] (g=c%4).
Each core computes its 3 heads' attention plus their partial contribution to
the final projection; the host sums the 4 partials per batch.

v2 redesign (vs v1):
- all matmuls bf16 (1 cyc/row at any moving size; no fp32r small-N penalty)
- v computed directly in [s, d] layout (no PE transposes); v bias folded into
  phase 3 via a constant ones row in ot2e and a host-precomputed bv@wf row
- causal mask added via PE matmul (ident^T @ mask) inside the S^T psum
  accumulation group instead of a DVE tensor_tensor
- exp instructions batched over pairs of key blocks (2-bank PSUM st tiles)
- DMA issue on sync + gpsimd queues (keeps Act SEQ free for exp)
- phase-3 ob moves split DVE (cols 0:512) / Act (cols 512:768)
- fine-grained emission interleave (attention vs projection/phase-3 filler)
  to keep the PE stream dense

Per-core program:
  phase 1: qT/kT = Wqk^T x^T + b, column groups [q0 q1][q2 k0][k1 k2];
           v[s,d] = x W_v^T per 128-query block (3 heads side by side)
  phase 2: per head h, key-block j: S^T_j = k_j^T q (K=64), +mask on diagonal
           blocks via matmul, P = exp(scale*S^T) (Act, pair-batched),
           [O^T; l] accumulated via matmul(lhsT=[v_j | 1], rhs=P).
           recip = 1/l (DVE), partition-broadcast via K=1 matmul, multiply.
  phase 3: out = [ot01; ot2e]^T @ [wf01; wf2e] -> [2048, 768] partial, DMA.

`repeat` unrolls the whole body N times in one NEFF; test.py measures
per-body HW time as the slope of wall time vs repeat count.
"""
import numpy as np

EMBED_DIM = 768
B = 2
T = 2048
N_CORES = 8
NT = T // 128           # 16 query/key tiles
SCALE = 1.0 / np.sqrt(64.0)
NEG = -1.0e9


_state = {}


def _build(repeat=1):
    import concourse.tile as tile
    from concourse import bacc, mybir
    from concourse.masks import make_identity

    F32 = mybir.dt.float32
    BF16 = mybir.dt.bfloat16

    nc = bacc.Bacc("TRN2", target_bir_lowering=False, debug=False)

    xT_d = nc.dram_tensor("xT", [EMBED_DIM, T], BF16, kind="ExternalInput").ap()
    # columns ordered [q0 q1 | k0 k1 | q2 k2]
    wqk_d = nc.dram_tensor("wqk", [EMBED_DIM, 384], BF16, kind="ExternalInput").ap()
    wv_d = nc.dram_tensor("wv", [EMBED_DIM, 192], BF16, kind="ExternalInput").ap()
    bqk_d = nc.dram_tensor("bqk", [384, 1], F32, kind="ExternalInput").ap()
    wf01_d = nc.dram_tensor("wf01", [128, EMBED_DIM], BF16, kind="ExternalInput").ap()
    wf2e_d = nc.dram_tensor("wf2e", [65, EMBED_DIM], BF16, kind="ExternalInput").ap()
    mask_d = nc.dram_tensor("mask", [128, 128], BF16, kind="ExternalInput").ap()
    out_d = nc.dram_tensor("out_p", [T, EMBED_DIM], BF16, kind="ExternalOutput").ap()

    with tile.TileContext(nc) as tc:
        with tc.tile_pool(name="const", bufs=1) as const, \
             tc.tile_pool(name="persist", bufs=1) as persist:
            # ---- constants ----
            wqk_sb = const.tile([128, 6, 384], BF16)
            wv_sb = const.tile([128, 6, 192], BF16)
            nc.sync.dma_start(out=wqk_sb[:], in_=wqk_d.rearrange("(k p) c -> p k c", p=128))
            nc.gpsimd.dma_start(out=wv_sb[:], in_=wv_d.rearrange("(k p) c -> p k c", p=128))
            bqk_sb = [const.tile([128, 1], F32, name=f"bqk{m}", tag=f"bqk{m}")
                      for m in range(3)]
            for m in range(3):
                nc.sync.dma_start(out=bqk_sb[m][:], in_=bqk_d[128 * m:128 * (m + 1), :])
            wf01_sb = const.tile([128, EMBED_DIM], BF16)
            wf2e_sb = const.tile([65, EMBED_DIM], BF16)
            nc.gpsimd.dma_start(out=wf01_sb[:], in_=wf01_d[:])
            nc.gpsimd.dma_start(out=wf2e_sb[:], in_=wf2e_d[:])
            mask_sb = const.tile([128, 128], BF16)
            nc.sync.dma_start(out=mask_sb[:], in_=mask_d[:])
            ident_f = const.tile([128, 128], F32)
            make_identity(nc, ident_f)
            ident_bf = const.tile([128, 128], BF16)
            nc.vector.tensor_copy(out=ident_bf[:], in_=ident_f[:])
            ones_bf = const.tile([65, 64], BF16)
            nc.vector.memset(ones_bf[:], 1.0)

            # ---- persistent activations ----
            qA = persist.tile([128, T], BF16)    # q0 @0:64, q1 @64:128
            kA = persist.tile([128, T], BF16)    # k0 @0:64, k1 @64:128
            qB = persist.tile([64, T], BF16)     # q2
            kB = persist.tile([64, T], BF16)     # k2
            v_all = persist.tile([128, NT, 3, 65], BF16)   # [v | 1] per head
            nc.vector.memset(v_all[:, :, :, 64:65], 1.0)
            ot01 = persist.tile([128, T], BF16)  # normalized O^T h0 (@0), h1 (@64)
            ot2e = persist.tile([65, T], BF16)   # h2 @0:64; row 64 = ones
            nc.vector.memset(ot2e[64:65, :], 1.0)

            with tc.tile_pool(name="sb", bufs=1) as sbp, \
                 tc.tile_pool(name="ps", bufs=1, space="PSUM") as psp:
                for rep in range(repeat):
                    _emit_body(nc, tc, rep, locals())

    nc.compile()
    return nc


def _emit_body(nc, tc, rep, env):
    """Emit one forward pass with fine-grained interleaving.

    PSUM tags (8 banks): ps0 ps1 (projection/v/bc groups), stp0 stp1
    (2-bank S^T pair tiles), otl0 otl1 (PV accumulators + phase-3)."""
    from concourse import mybir

    F32 = mybir.dt.float32
    BF16 = mybir.dt.bfloat16
    Exp = mybir.ActivationFunctionType.Exp
    MULT = mybir.AluOpType.mult

    xT_d, out_d = env["xT_d"], env["out_d"]
    wqk_sb, wv_sb = env["wqk_sb"], env["wv_sb"]
    bqk_sb = env["bqk_sb"]
    wf01_sb, wf2e_sb = env["wf01_sb"], env["wf2e_sb"]
    ident_bf, ones_bf, mask_sb = env["ident_bf"], env["ones_bf"], env["mask_sb"]
    qA, kA, qB, kB = env["qA"], env["kA"], env["qB"], env["kB"]
    v_all = env["v_all"]
    ot01, ot2e = env["ot01"], env["ot2e"]
    dmae = [nc.sync, nc.gpsimd]
    sbp, psp = env["sbp"], env["psp"]

    if True:
        # ---- input DMA: xT as 6x4 chunks spread over 2 DMA queues ----
        xT_t = [[sbp.tile([128, 512], BF16, name=f"xT{rep}_{k}_{n}",
                          tag=f"xT{k}{n}") for n in range(4)] for k in range(6)]
        di = 0
        for n in range(4):
            for k in range(6):
                dmae[di % 2].dma_start(
                    out=xT_t[k][n][:],
                    in_=xT_d[128 * k:128 * (k + 1), 512 * n:512 * (n + 1)])
                di += 1

        gidx = [0]

        def qk_group(m, n):
            ps = psp.tile([128, 512], F32, name=f"pg{rep}_{gidx[0]}",
                          tag=f"ps{gidx[0] % 2}")
            gidx[0] += 1
            for k in range(6):
                nc.tensor.matmul(ps[:], lhsT=wqk_sb[:, k, 128 * m:128 * (m + 1)],
                                 rhs=xT_t[k][n][:], start=(k == 0), stop=(k == 5))
            nsl = slice(512 * n, 512 * (n + 1))
            if m < 2:
                dst = qA if m == 0 else kA
                nc.vector.tensor_scalar_add(out=dst[:, nsl], in0=ps[:],
                                            scalar1=bqk_sb[m][:])
            else:
                nc.vector.tensor_scalar_add(out=qB[:, nsl], in0=ps[0:64, :],
                                            scalar1=bqk_sb[2][0:64, :])
                nc.vector.tensor_scalar_add(out=kB[:, nsl], in0=ps[64:128, :],
                                            scalar1=bqk_sb[2][64:128, :])

        def v_block(i):
            # v[s, d] for s-block i, 3 heads side by side: [128, 192]
            n, off = divmod(128 * i, 512)
            ps = psp.tile([128, 512], F32, name=f"vp{rep}_{i}",
                          tag=f"ps{gidx[0] % 2}")
            gidx[0] += 1
            for k in range(6):
                nc.tensor.matmul(ps[:, 0:192], lhsT=xT_t[k][n][:, off:off + 128],
                                 rhs=wv_sb[:, k, :], start=(k == 0), stop=(k == 5))
            nc.vector.tensor_copy(
                out=v_all[:, i, :, 0:64],
                in_=ps[:, 0:192].rearrange("p (h d) -> p h d", h=3))

        # h -> (qT tile, q part offset, kT tile, k part offset)
        head_cfg = [(qA, 0, kA, 0), (qA, 64, kA, 64), (qB, 0, kB, 0)]
        sidx = [0]
        otli = [0]
        attn_st = {}

        def attn_S(h, q, inject=None):
            """S^T matmuls + diag mask + pair-batched exp for quarter q.
            inject: {pair_index: item} emitted after that pair, to absorb the
            exp pipeline phase lag without displacing the S stream."""
            qT, oq, kT, ok = head_cfg[h]
            base = 512 * q
            pairs = []
            for p in range(2 * q + 2):
                if inject and p in inject:
                    inject.pop(p)()
                st2 = psp.tile([128, 2, 512], F32, name=f"st{rep}_{h}{q}{p}",
                               tag=f"stp{sidx[0] % 2}")
                pt2 = sbp.tile([128, 2, 512], BF16, name=f"pt{rep}_{h}{q}{p}",
                               tag=f"pt{sidx[0] % 6}")
                sidx[0] += 1
                lns = []
                diags = []
                for jj in range(2):
                    j = 2 * p + jj
                    s0 = max(base, 128 * j)
                    ln = base + 512 - s0
                    lns.append(ln)
                    if 128 * j >= base:
                        diags.append(jj)
                    nc.tensor.matmul(
                        st2[:, jj, 0:ln],
                        lhsT=kT[ok:ok + 64, 128 * j:128 * (j + 1)],
                        rhs=qT[oq:oq + 64, s0:s0 + ln],
                        start=True, stop=True)
                mx = max(lns)
                nc.scalar.activation(out=pt2[:, :, 0:mx], in_=st2[:, :, 0:mx],
                                     func=Exp, scale=float(SCALE))
                for jj in diags:
                    nc.vector.tensor_tensor(
                        out=pt2[:, jj, 0:128], in0=pt2[:, jj, 0:128],
                        in1=mask_sb[:], op=MULT)
                pairs.append((st2, pt2, lns))
            attn_st[(h, q)] = pairs

        def attn_PV(h, q):
            base = 512 * q
            pairs = attn_st.pop((h, q))
            otl = psp.tile([128, 512], F32, name=f"otl{rep}_{h}{q}",
                           tag=f"otl{otli[0] % 2}")
            otli[0] += 1
            jmax = 4 * q + 3
            for j in range(jmax + 1):
                s0 = max(base, 128 * j)
                ln = base + 512 - s0
                pt2 = pairs[j // 2][1]
                nc.tensor.matmul(
                    otl[0:65, s0 - base:512],
                    lhsT=v_all[:, j, h, :], rhs=pt2[:, j % 2, 0:ln],
                    start=(j == 0), stop=(j == jmax))
            return otl

        def attn_norm(h, q, otl):
            base = 512 * q
            recip = sbp.tile([65, 512], F32, name=f"rc{rep}_{h}{q}",
                             tag=f"rc{(3 * q + h) % 2}")
            nc.vector.reciprocal(out=recip[0:1, :], in_=otl[64:65, :])
            bcs = sbp.tile([64, 512], F32, name=f"bcs{rep}_{h}{q}",
                           tag=f"bcs{(3 * q + h) % 2}")
            nc.gpsimd.partition_broadcast(bcs[:, :], recip[0:1, :])
            dst, od = (ot2e, 0) if h == 2 else (ot01, 64 * h)
            nc.vector.tensor_tensor(
                out=dst[od:od + 64, base:base + 512],
                in0=otl[0:64, :], in1=bcs[:], op=MULT)

        p3_ob = {}

        def phase3a(i):
            fpa = psp.tile([128, 512], F32, name=f"fpa{rep}_{i}", tag=f"otl{i % 2}")
            ti = slice(128 * i, 128 * (i + 1))
            nc.tensor.matmul(fpa[:], lhsT=ot01[:, ti],
                             rhs=wf01_sb[:, 0:512], start=True, stop=False)
            nc.tensor.matmul(fpa[:], lhsT=ot2e[:, ti],
                             rhs=wf2e_sb[:, 0:512], start=False, stop=True)
            ob = sbp.tile([128, EMBED_DIM], BF16, name=f"ob{rep}_{i}",
                          tag=f"ob{i % 6}")
            p3_ob[i] = ob
            nc.vector.tensor_copy(out=ob[:, 0:512], in_=fpa[:, :])

        def phase3b(i):
            fpb = psp.tile([128, 512], F32, name=f"fpb{rep}_{i}", tag=f"otl{i % 2}")
            ti = slice(128 * i, 128 * (i + 1))
            nc.tensor.matmul(fpb[:, 0:256], lhsT=ot01[:, ti],
                             rhs=wf01_sb[:, 512:768], start=True, stop=False)
            nc.tensor.matmul(fpb[:, 0:256], lhsT=ot2e[:, ti],
                             rhs=wf2e_sb[:, 512:768], start=False, stop=True)
            ob = p3_ob.pop(i)
            nc.vector.tensor_copy(out=ob[:, 512:768], in_=fpb[:, 0:256])
            dmae[i % 2].dma_start(out=out_d[ti, :], in_=ob[:])

        # ---- staged emission ----
        # Safety rule: attention quarters are emitted as contiguous blocks
        # (their st/pt/otl tag rotations + cross-engine deps form cycles if
        # other otl-tag users are woven in).  Projections and phase-3 use
        # disjoint tag sets, so they can interleave with each other freely.
        def attn_quarter(q, prelude=(), inject=None):
            """Attention for quarter q, inter-head pipelined.  `prelude` items
            (phase-3 leftovers) are emitted after the first S batch so their
            psum-tag waits hide behind this quarter's exp latency."""
            otl = {}
            attn_S(0, q, inject)
            for it in prelude:
                it()
            attn_S(1, q)
            otl[0] = attn_PV(0, q)
            attn_S(2, q)
            otl[1] = attn_PV(1, q)
            attn_norm(0, q, otl.pop(0))
            otl[2] = attn_PV(2, q)
            attn_norm(1, q, otl.pop(1))
            attn_norm(2, q, otl.pop(2))

        def proj_items(n):
            items = [lambda m=m: qk_group(m, n) for m in range(3)]
            items += [lambda i=i: v_block(i) for i in range(4 * n, 4 * n + 4)]
            return items

        def phase3_items(i0, i1, defer_tail=False):
            """a/b sub-items pipelined: a(i), a(i+1), b(i), a(i+2), b(i+1)...
            With defer_tail, the trailing b-items are split off for the
            caller to emit later."""
            out = []
            pend = []
            for i in range(i0, i1):
                out.append(lambda i=i: phase3a(i))
                pend.append(lambda i=i: phase3b(i))
                if len(pend) > 1:
                    out.append(pend.pop(0))
            if defer_tail:
                return out, pend
            out.extend(pend)
            return out

        def interleave(a_items, b_items):
            out = []
            na, nb = len(a_items), len(b_items)
            bi = 0
            for ai, item in enumerate(a_items):
                out.append(item)
                want = int(round((ai + 1) * nb / na))
                while bi < want:
                    out.append(b_items[bi])
                    bi += 1
            out.extend(b_items[bi:])
            return out

        for it in proj_items(0) + proj_items(1):
            it()
        attn_quarter(0)
        p2 = proj_items(2)
        vb8 = p2.pop(3)
        for it in p2:
            it()
        attn_quarter(1, inject={2: vb8})
        p3i = proj_items(3)
        vb12 = p3i.pop(3)
        for it in interleave(p3i, phase3_items(0, 4)):
            it()
        attn_quarter(2, inject={2: vb12})
        items, tail = phase3_items(4, 8, defer_tail=True)
        for it in items:
            it()
        attn_quarter(3, prelude=tail + phase3_items(8, 12))
        for it in phase3_items(12, 16):
            it()
